# revision 1
# baseline (speedup 1.0000x reference)
"""Trainium2 Bass kernel for GQA causal sliding-window self-attention.

Problem: B=2, T=2048, C=1024, 16 heads (hd=64), 4 KV groups, window=256.
  q = x@Wq+bq; k = x@Wk+bk; v = x@Wv+bv  (GQA repeat of kv over 4 heads)
  att = softmax(mask(q k^T / 8));  y = (att v) @ Wo + bo

Sharding: data-parallel over (batch, T-chunk). 8 cores = 2 batches x 4
chunks of 512 query tokens. Each core receives the 768-token extended
x slice (512 queries + 256 halo for the window) and computes its chunk's
output rows completely locally; no collectives.

Per-core layout: everything is computed "transposed" (feature dim on
partitions) so the TensorE contraction dim is always on partitions and
softmax denominators come out of the att@v matmul for free via a
ones-column appended to V:
  xT [1024c, 768t] -> qT [1024, 512], kT (per-group, duplicated into both
  64-partition halves so lhsT/rhs base partitions match) [128, 768],
  v token-major [128t, 4g, 64+1].
  scoresT[kj, qi] blocks of [512kj, 256qi] per (head, q-block), split
  into two 1-bank psum tiles for pipelining; exp on ScalarE straight
  from psum, then a multiplicative 0/1 band mask as an SBUF-only DVE
  multiply (scores are bounded, so exp cannot overflow before masking);
  att@v gives yT_aug [65, 256] whose row 64 is the softmax
  denominator. Normalization: DVE reciprocal of the denominator row, K=1
  broadcast matmul on PE, staging copies on ScalarE, and the multiply on
  the otherwise-idle GPSIMD engine. The final projection uses yT as lhsT
  so the output psum is token-major and is DMA'd straight out.

All matmuls run in float32r (fp32 storage, 1 cycle/row at N>=256; every
producer of a matmul operand declares an fp32r output view).

Schedule: few big HWDGE DMAs ordered by consumption deadline; Wq is
passed (m, kc)-tiled so each head-pair's q projection starts as soon as
its 0.5 MB arrives; q-block-0 attention runs interleaved with the q
projections, and q-block-0's output projection is interleaved into
q-block-1's attention so PE always has fill-in work.

Host folds 1/sqrt(64) into Wq/bq and applies the exactly-linear bv/bo
corrections after the device pass:  out += bv_rep @ Wo + bo.

Environment workarounds (this container's walrus build): max 1 sync wait
per CTRL/fp32r-matmul instruction (see _split_multi_waits and the
chunked tail drain), no fp32r memset, no custom gpsimd ISA ops.
"""
import sys

sys.path.insert(0, "/opt/trn_rl_repo")

import contextlib

import numpy as np

import concourse.bass as bass
import concourse.tile as tile
from concourse import mybir
from concourse.bass_utils import run_bass_kernel_spmd
from concourse import library_config
from concourse.vector_clock import ScopedClock

F32 = mybir.dt.float32
F32R = mybir.dt.float32r

B, T, C = 2, 2048, 1024
NH, NG, HD = 16, 4, 64
KV = NG * HD  # 256
WINDOW = 256
NCORES = 8
TQ = 512  # query tokens per core
TE = TQ + WINDOW  # 768 extended tokens per core
NEG = -1e9
KC = C // 128  # 8 contraction tiles


class _ChunkedDrainTileContext(tile.TileContext):
    """Walrus in this container only accepts 1 sync wait on CTRL-class
    instructions; the stock Tile tail drain carries one wait per
    outstanding proc. Spread them over SP nops first, and use the cheaper
    sem-only barriers for the tail."""

    def _drain_and_barrier(self, tick_clock, wait_clock):
        gc = tick_clock.global_clock
        entries = []
        for scope, vc in ScopedClock({None: gc}).items():
            for proc in range(len(vc)):
                t = vc[proc]
                if t > 0:
                    entries.append((scope, proc, t))
        # Spread the one-wait-per-instruction tail waits across engines so
        # they resolve in parallel; the sem-only barrier then syncs engines.
        engines = [self.nc.sync, self.nc.vector, self.nc.scalar, self.nc.gpsimd]
        curs = [ScopedClock() for _ in engines]
        for i, (scope, proc, t) in enumerate(entries):
            eng = engines[i % len(engines)]
            nop = eng.nop(nofuse=True, hint="tail_wait")
            partial = ScopedClock()
            partial.require_at_least(scope, proc, t)
            wait_clock.add_sem_waits(nop.ins, partial, curs[i % len(engines)])
            curs[i % len(engines)].update_past(partial)
        self.nc.all_engine_barrier(sem_only=True)
        drain_inst = self.nc.sync.drain()
        cur = ScopedClock()
        for c in curs:
            cur.update_past(c)
        wait_clock.add_sem_waits(drain_inst.ins, ScopedClock({None: gc}), cur)
        assert self.sems is not None
        popped = self.nc._tile_sem_poison_stack.pop()
        assert popped is self._sem_poison
        self.nc.clear_and_free_semaphores(list(self.sems.allocated().values()))


def _r(ap):
    return ap.bitcast(F32R)


def _split_multi_waits(nc, max_waits=1):
    """This walrus build rejects >1 sync wait on several instruction structs
    (CTRL, self-loading fp32r Matmult). Hoist excess waits onto same-engine
    NOPs placed immediately before the instruction — identical semantics."""
    fn = nc.m.functions[0]
    for blk in fn.blocks:
        insts = blk.instructions
        new = []
        changed = False
        for inst in insts:
            si = inst.sync_info
            waits = list(si.on_wait) if si is not None and si.on_wait else []
            if len(waits) > max_waits:
                changed = True
                for w in waits[:-max_waits]:
                    nop = mybir.InstNoOp(
                        name=nc.get_next_instruction_name(),
                        ins=[],
                        outs=[],
                        engine=inst.engine,
                        sync_info=mybir.SyncInfo(on_wait=[w], on_update=[]),
                        bass_nofuse=True,
                    )
                    nc.register_instruction(nop, overwrite=True)
                    new.append(nop)
                si.on_wait = waits[-max_waits:]
                inst.sync_info = si
            new.append(inst)
        if changed:
            blk.instructions = new


def _build_program():
    nc = bass.Bass("TRN2", target_bir_lowering=False, debug=False, num_devices=NCORES)

    xt = nc.dram_tensor("xt", [128, KC, TE], F32, kind="ExternalInput")
    wq = nc.dram_tensor("wq", [KC, 128, KC, 128], F32, kind="ExternalInput")  # [m][p][kc]
    wk = nc.dram_tensor("wk", [128, KC, KV], F32, kind="ExternalInput")
    wv = nc.dram_tensor("wv", [128, KC, KV], F32, kind="ExternalInput")
    wo = nc.dram_tensor("wo", [128, KC, C], F32, kind="ExternalInput")
    bq = nc.dram_tensor("bq", [C, 1], F32, kind="ExternalInput")
    bk = nc.dram_tensor("bk", [KV, 1], F32, kind="ExternalInput")
    maskp = nc.dram_tensor("maskp", [128, 4, TQ], F32, kind="ExternalInput")
    out = nc.dram_tensor("out", [TQ, C], F32, kind="ExternalOutput")

    with _ChunkedDrainTileContext(nc) as tc:
        with contextlib.ExitStack() as ctx:
            wsb = ctx.enter_context(tc.tile_pool(name="wsb", bufs=1))
            xsb = ctx.enter_context(tc.tile_pool(name="xsb", bufs=1))
            csb = ctx.enter_context(tc.tile_pool(name="csb", bufs=1))
            qkv = ctx.enter_context(tc.tile_pool(name="qkv", bufs=1))
            ynp = ctx.enter_context(tc.tile_pool(name="ynp", bufs=1))
            expp = ctx.enter_context(tc.tile_pool(name="expp", bufs=3))
            rrp = ctx.enter_context(tc.tile_pool(name="rrp", bufs=4))
            outp = ctx.enter_context(tc.tile_pool(name="outp", bufs=2))
            ytsp = ctx.enter_context(tc.tile_pool(name="ytsp", bufs=3))
            rbp = ctx.enter_context(tc.tile_pool(name="rbp", bufs=3))
            pj = ctx.enter_context(tc.tile_pool(name="pj", bufs=3, space="PSUM"))
            scp_pool = ctx.enter_context(tc.tile_pool(name="scp", bufs=3, space="PSUM"))
            ytp_pool = ctx.enter_context(tc.tile_pool(name="ytp", bufs=2, space="PSUM"))

            # ---- loads (few big DMAs, ordered by consumption deadline) ----
            wk_all = wsb.tile([128, KC, KV], F32, name="wk_all", tag="wk_all")
            nc.sync.dma_start(out=_r(wk_all[:]), in_=_r(wk[:]))
            xt_all = xsb.tile([128, KC, TE], F32, name="xt_all", tag="xt_all")
            nc.sync.dma_start(out=_r(xt_all[:, 0:2, :]), in_=_r(xt[:, 0:2, :]))
            wv_all = wsb.tile([128, KC, KV], F32, name="wv_all", tag="wv_all")
            nc.sync.dma_start(out=_r(wv_all[:]), in_=_r(wv[:]))
            for q4 in range(1, 4):
                nc.sync.dma_start(
                    out=_r(xt_all[:, 2 * q4 : 2 * q4 + 2, :]),
                    in_=_r(xt[:, 2 * q4 : 2 * q4 + 2, :]),
                )
            bq_all = csb.tile([128, KC], F32)
            nc.sync.dma_start(out=bq_all[:], in_=bq[:, 0].rearrange("(m p) -> p m", p=128))
            bk_all = csb.tile([128, 2], F32)
            nc.sync.dma_start(out=bk_all[:], in_=bk[:, 0].rearrange("(m p) -> p m", p=128))
            ones64_f = csb.tile([1, 64], F32)
            nc.vector.memset(ones64_f[:], 1.0)
            ones64 = csb.tile([1, 64], F32)
            nc.vector.tensor_copy(_r(ones64[:]), ones64_f[:])
            onescol_f = csb.tile([128, NG, 1], F32)
            nc.vector.memset(onescol_f[:], 1.0)

            # Wq arrives (m, kc)-tiled so head-pair m can start after 0.5 MB.
            wq_sb = [None] * KC

            def _load_wq(m):
                t = wsb.tile([128, KC, 128], F32, name=f"wq{m}", tag=f"wq{m}")
                nc.sync.dma_start(out=_r(t[:]), in_=_r(wq[m, :, :, :]))
                wq_sb[m] = t

            for m in range(2):
                _load_wq(m)
            mask_sb = csb.tile([128, 4, TQ], F32)
            nc.sync.dma_start(out=mask_sb[:], in_=maskp[:])
            for m in range(2, KC):
                _load_wq(m)
            wo_all = wsb.tile([128, KC, C], F32, name="wo_all", tag="wo_all")
            nc.sync.dma_start(out=_r(wo_all[:]), in_=_r(wo[:]))

            # ---- kT projection: kT[g][dup_half*64+d, te] ----
            kT_sb = [qkv.tile([128, TE], F32, name=f"kT{g}", tag=f"kT{g}") for g in range(NG)]
            for mt in range(2):  # kv partition tile (2 groups each)
                for s2 in range(2):  # token span halves of 384
                    kp = pj.tile([128, 512], F32, name="kp", tag="pj")
                    for kc in range(KC):
                        nc.tensor.matmul(
                            kp[:, 0:384],
                            _r(wk_all[:, kc, mt * 128 : (mt + 1) * 128]),
                            _r(xt_all[:, kc, s2 * 384 : (s2 + 1) * 384]),
                            start=(kc == 0),
                            stop=(kc == KC - 1),
                        )
                    for gh in range(2):  # source half (group g = 2*mt+gh)
                        g = 2 * mt + gh
                        for half in range(2):  # dest duplicated half
                            nc.vector.tensor_scalar_add(
                                _r(kT_sb[g][half * 64 : half * 64 + 64, s2 * 384 : (s2 + 1) * 384]),
                                kp[gh * 64 : gh * 64 + 64, 0:384],
                                bk_all[gh * 64 : gh * 64 + 64, mt : mt + 1],
                            )

            # ---- v projection: token-major with ones column ----
            v_sb = [qkv.tile([128, NG, HD + 1], F32, name=f"v{vt}", tag=f"v{vt}") for vt in range(6)]
            for vt in range(6):
                vp = pj.tile([128, 512], F32, name="vp", tag="pj")
                for kc in range(KC):
                    nc.tensor.matmul(
                        vp[:, 0:KV],
                        _r(xt_all[:, kc, vt * 128 : (vt + 1) * 128]),
                        _r(wv_all[:, kc, :]),
                        start=(kc == 0),
                        stop=(kc == KC - 1),
                    )
                nc.scalar.copy(
                    _r(v_sb[vt][:, :, 0:HD]),
                    vp[:, 0:KV].rearrange("p (g d) -> p g d", g=NG),
                )
                nc.scalar.copy(_r(v_sb[vt][:, :, HD : HD + 1]), onescol_f[:])

            yn_sb = [ynp.tile([128, TQ], F32, name=f"yn{m}", tag=f"yn{m}") for m in range(KC)]
            qT_sb = [None] * KC

            def attn_block(h, qb, qT):
                hh, g = h % 2, h // 4
                m = h // 2
                scpA = scp_pool.tile([128, 2, 256], F32, name="scpA", tag="sc")
                scpB = scp_pool.tile([128, 2, 256], F32, name="scpB", tag="sc")
                halves = (scpA, scpB)
                for kt in range(4):
                    ke0 = qb * 256 + kt * 128
                    nc.tensor.matmul(
                        halves[kt // 2][:, kt % 2, :],
                        _r(kT_sb[g][hh * 64 : hh * 64 + 64, ke0 : ke0 + 128]),
                        _r(qT[hh * 64 : hh * 64 + 64, qb * 256 : qb * 256 + 256]),
                        start=True,
                        stop=True,
                    )
                ex = expp.tile([128, 4, 256], F32, name="ex", tag="ex")
                for half in range(2):
                    nc.scalar.activation(
                        _r(ex[:, 2 * half : 2 * half + 2, :]),
                        halves[half][:],
                        mybir.ActivationFunctionType.Exp,
                    )
                    nc.vector.tensor_tensor(
                        _r(ex[:, 2 * half : 2 * half + 2, :]),
                        ex[:, 2 * half : 2 * half + 2, :],
                        mask_sb[:, 2 * half : 2 * half + 2, qb * 256 : qb * 256 + 256],
                        mybir.AluOpType.mult,
                    )
                ytp = ytp_pool.tile([HD + 1, 256], F32, name="ytp", tag="yt")
                for kt in range(4):
                    vt = qb * 2 + kt
                    nc.tensor.matmul(
                        ytp[:],
                        _r(v_sb[vt][:, g, :]),
                        _r(ex[:, kt, :]),
                        start=(kt == 0),
                        stop=(kt == 3),
                    )
                yts = ytsp.tile([HD, 256], F32, name="yts", tag="yts")
                nc.scalar.copy(yts[:], ytp[0:HD, :])
                rr = rrp.tile([1, 256], F32, name="rr", tag="rr")
                with nc.allow_low_precision(reason="softmax denom reciprocal in fp32r"):
                    nc.vector.reciprocal(_r(rr[:]), ytp[HD : HD + 1, :])
                sp = ytp_pool.tile([HD + 1, 256], F32, name="sp", tag="yt")
                nc.tensor.matmul(
                    sp[0:64, 0:256], _r(ones64[:]), _r(rr[:]), start=True, stop=True
                )
                ssb = rbp.tile([HD, 256], F32, name="ssb", tag="ssb")
                nc.scalar.copy(ssb[:], sp[0:64, 0:256])
                nc.gpsimd.tensor_tensor(
                    _r(yn_sb[m][hh * 64 : hh * 64 + 64, qb * 256 : qb * 256 + 256]),
                    yts[:],
                    ssb[:],
                    mybir.AluOpType.mult,
                )

            def out_proj(tt):
                ob = outp.tile([128, C], F32, name="ob", tag="ob")
                for n2 in range(2):
                    op = pj.tile([128, 512], F32, name="op", tag="pj")
                    for kc in range(KC):
                        nc.tensor.matmul(
                            op[:],
                            _r(yn_sb[kc][:, tt * 128 : (tt + 1) * 128]),
                            _r(wo_all[:, kc, n2 * 512 : (n2 + 1) * 512]),
                            start=(kc == 0),
                            stop=(kc == KC - 1),
                        )
                    nc.scalar.copy(ob[:, n2 * 512 : (n2 + 1) * 512], op[:])
                nc.sync.dma_start(out=out[tt * 128 : (tt + 1) * 128, :], in_=ob[:])

            # q-block 0: project qT per head pair, attention immediately
            for m in range(KC):
                qp = pj.tile([128, 512], F32, name="qp", tag="pj")
                for kc in range(KC):
                    nc.tensor.matmul(
                        qp[:],
                        _r(wq_sb[m][:, kc, :]),
                        _r(xt_all[:, kc, WINDOW:TE]),
                        start=(kc == 0),
                        stop=(kc == KC - 1),
                    )
                qT = qkv.tile([128, TQ], F32, name=f"qT{m}", tag=f"qT{m}")
                nc.vector.tensor_scalar_add(_r(qT[:]), qp[:], bq_all[:, m : m + 1])
                qT_sb[m] = qT
                attn_block(2 * m, 0, qT)
                attn_block(2 * m + 1, 0, qT)

            # q-block 1 attention with q-block-0 output projection interleaved
            for m in range(KC):
                attn_block(2 * m, 1, qT_sb[m])
                attn_block(2 * m + 1, 1, qT_sb[m])
                if m == 1:
                    out_proj(0)
                if m == 5:
                    out_proj(1)
            out_proj(2)
            out_proj(3)

    _split_multi_waits(nc)
    return nc


_NC = None


def _get_nc():
    global _NC
    if _NC is None:
        _NC = _build_program()
    return _NC


def _host_prep(x, Wq, bq, Wk, bk, Wv, bv, Wo, bo):
    x = np.ascontiguousarray(np.asarray(x, dtype=np.float32))
    Wq = np.asarray(Wq, np.float32)
    bq = np.asarray(bq, np.float32)
    Wk = np.asarray(Wk, np.float32)
    bk = np.asarray(bk, np.float32)
    Wv = np.asarray(Wv, np.float32)
    bv = np.asarray(bv, np.float32)
    Wo = np.asarray(Wo, np.float32)
    bo = np.asarray(bo, np.float32)

    scale = np.float32(1.0 / np.sqrt(HD))
    # (m, kc)-tiled, pre-scaled Wq: wq_t[m, kc] = Wq[kc-tile, m-tile] * scale
    wq_t = np.ascontiguousarray(
        (Wq * scale).reshape(KC, 128, KC, 128).transpose(2, 1, 0, 3)
    )
    bq_h = np.ascontiguousarray((bq * scale).reshape(C, 1))
    wk_h = np.ascontiguousarray(Wk.reshape(KC, 128, KV).transpose(1, 0, 2))
    wv_h = np.ascontiguousarray(Wv.reshape(KC, 128, KV).transpose(1, 0, 2))
    wo_h = np.ascontiguousarray(Wo.reshape(KC, 128, C).transpose(1, 0, 2))
    bk_h = np.ascontiguousarray(bk.reshape(KV, 1))

    # band masks, block-packed: maskp[p, kt, qb*256+qi] for kj_l = kt*128+p
    kj = np.arange(2 * WINDOW)[:, None]  # 512 local k indices within a q-block
    qi = np.arange(WINDOW)[None, :]  # 256 local q indices within a q-block
    band = (qi <= kj) & (kj <= qi + WINDOW)  # same for every block
    masks = {}
    for c in range(4):
        mk = np.empty((128, 4, TQ), np.float32)
        for qb in range(2):
            valid = band.copy()
            if c == 0 and qb == 0:
                valid &= kj >= WINDOW  # global j >= 0 at the sequence start
            mkb = np.where(valid, np.float32(1.0), np.float32(0.0))
            mk[:, :, qb * 256 : (qb + 1) * 256] = (
                mkb.reshape(4, 128, WINDOW).transpose(1, 0, 2)
            )
        masks[c] = mk

    in_maps = []
    for core in range(NCORES):
        b, c = core // 4, core % 4
        t0 = c * TQ - WINDOW
        xe = np.zeros((TE, C), np.float32)
        lo = max(t0, 0)
        xe[lo - t0 : TE, :] = x[b, lo : t0 + TE, :]
        in_maps.append(
            {
                "xt": np.ascontiguousarray(xe.T.reshape(KC, 128, TE).transpose(1, 0, 2)),
                "wq": wq_t,
                "wk": wk_h,
                "wv": wv_h,
                "wo": wo_h,
                "bq": bq_h,
                "bk": bk_h,
                "maskp": masks[c],
            }
        )

    # exact linear bias correction applied host-side:
    # y = att@(v+bv) = att@v + bv (softmax rows sum to 1), so
    # out += bv_rep @ Wo + bo
    bv_rep = np.concatenate([bv[(h // NG) * HD : (h // NG + 1) * HD] for h in range(NH)])
    corr = bv_rep.astype(np.float64) @ Wo.astype(np.float64) + bo.astype(np.float64)
    return in_maps, corr.astype(np.float32)


LAST_RESULTS = None


def kernel(x, Wq, bq, Wk, bk, Wv, bv, Wo, bo):
    global LAST_RESULTS
    in_maps, corr = _host_prep(x, Wq, bq, Wk, bk, Wv, bv, Wo, bo)
    nc = _get_nc()
    res = run_bass_kernel_spmd(nc, in_maps, core_ids=list(range(NCORES)))
    LAST_RESULTS = res
    out = np.empty((B, T, C), np.float32)
    for core in range(NCORES):
        b, c = core // 4, core % 4
        out[b, c * TQ : (c + 1) * TQ, :] = res.results[core]["out"]
    out += corr[None, None, :]
    return out



# revision 3
# speedup vs baseline: 1.0122x; 1.0122x over previous
"""Trainium2 Bass kernel for GQA causal sliding-window self-attention.

Problem: B=2, T=2048, C=1024, 16 heads (hd=64), 4 KV groups, window=256.

Sharding: data-parallel over (batch, T-chunk): 8 cores = 2 batches x 4
chunks of 512 query tokens; each core gets a 768-token extended x slice
(512 queries + 256 halo) and computes its output rows locally.

Design: all-bf16 operands (halves DMA, 1 cyc/col matmuls at any free
size), diagonal 128-query attention blocks (384-key span = 1.5x
overcompute instead of 2x), 2-head-packed score matmuls (qT stored
[64, 2, TQ] per head-pair, kT [64, NG, TE], all matmul operands at base
partition 0), TRANSPOSED av (queries on partitions; ones column in v
gives softmax denominators) so the reciprocal is per-partition and
normalization is one stride-0-broadcast DVE multiply, then a transpose
back to head-major via a plain bf16 matmul against an identity matrix.
Band masks are multiplicative 0/1 bf16 constants (GPSIMD kt0 +
qb0-edge kt1; DVE kt2; the middle kt tile needs no mask), applied to
the exp'd scores. Output is bf16, upcast on host; host folds 1/8 into
Wq/bq and adds the exact linear bv/bo correction (bv_rep @ Wo + bo).

PSUM (8 banks): scores [128,3,4,128] x2 bufs (6), combined
y_t[128,4,65]+bf16 transpose region x1 (1), shared proj/out bank (1).
Early projections borrow the scores pool before attention starts.

Pipeline: per unit (group, 128-query block): scores -> exp (ScalarE)
-> masks -> av -> recip/norm (DVE) -> transpose -> yn copy, with
q/v-projection and out-projection chunks interleaved as PE filler.

Environment constraints found the hard way (this walrus/axon build):
 - max 1 sync wait per instruction (_split_multi_waits hoists extras
   onto same-engine NOPs)
 - bf16 matmuls with operands at base partition 64 crash the compile;
   all operands must sit at partition 0 (hence qT/kT layouts)
 - is_transpose=True matmuls return wrong data on hw (interp-only)
 - only one open PSUM accumulation group per 2KB bank
 - GPSIMD cannot touch PSUM; DVE/ScalarE cross-partition-offset ok
 - DMA cannot read PSUM
"""

import sys

sys.path.insert(0, "/opt/trn_rl_repo")

import contextlib

import numpy as np

import concourse.bass as bass
import concourse.tile as tile
from concourse import mybir
from concourse.bass_utils import run_bass_kernel_spmd
from concourse.vector_clock import ScopedClock

F32 = mybir.dt.float32
BF16 = mybir.dt.bfloat16
F8E4 = mybir.dt.float8e4
FP8_SCALE = 32.0  # weights *32 into fp8 normal range; q carries the 1/32

import os

USE_IS_TRANSPOSE = os.environ.get("KV2_ISTR", "") == "1"  # wrong results on hw
USE_BCAST = os.environ.get("KV2_NO_BCAST", "") != "1"
USE_SC_IDBIAS = os.environ.get("KV2_NO_IDBIAS", "") != "1"

B, T, C = 2, 2048, 1024
NH, NG, HD = 16, 4, 64
KV = NG * HD  # 256
WINDOW = 256
NCORES = 8
TQ = 512  # query tokens per core
TE = TQ + WINDOW  # 768 extended tokens per core
KC = C // 128  # 8 contraction tiles


class _ChunkedDrainTileContext(tile.TileContext):
    """Walrus in this container only accepts 1 sync wait on CTRL-class
    instructions; spread the tail drain waits over engine NOPs."""

    def _drain_and_barrier(self, tick_clock, wait_clock):
        gc = tick_clock.global_clock
        entries = []
        for scope, vc in ScopedClock({None: gc}).items():
            for proc in range(len(vc)):
                t = vc[proc]
                if t > 0:
                    entries.append((scope, proc, t))
        engines = [self.nc.sync, self.nc.vector, self.nc.scalar, self.nc.gpsimd]
        curs = [ScopedClock() for _ in engines]
        for i, (scope, proc, t) in enumerate(entries):
            eng = engines[i % len(engines)]
            nop = eng.nop(nofuse=True, hint="tail_wait")
            partial = ScopedClock()
            partial.require_at_least(scope, proc, t)
            wait_clock.add_sem_waits(nop.ins, partial, curs[i % len(engines)])
            curs[i % len(engines)].update_past(partial)
        self.nc.all_engine_barrier(sem_only=True)
        drain_inst = self.nc.sync.drain()
        cur = ScopedClock()
        for c in curs:
            cur.update_past(c)
        wait_clock.add_sem_waits(drain_inst.ins, ScopedClock({None: gc}), cur)
        assert self.sems is not None
        popped = self.nc._tile_sem_poison_stack.pop()
        assert popped is self._sem_poison
        self.nc.clear_and_free_semaphores(list(self.sems.allocated().values()))


def _split_multi_waits(nc, max_waits=1):
    """Hoist excess sync waits onto same-engine NOPs (walrus quirk)."""
    fn = nc.m.functions[0]
    for blk in fn.blocks:
        insts = blk.instructions
        new = []
        changed = False
        for inst in insts:
            si = inst.sync_info
            waits = list(si.on_wait) if si is not None and si.on_wait else []
            if len(waits) > max_waits:
                changed = True
                for w in waits[:-max_waits]:
                    nop = mybir.InstNoOp(
                        name=nc.get_next_instruction_name(),
                        ins=[],
                        outs=[],
                        engine=inst.engine,
                        sync_info=mybir.SyncInfo(on_wait=[w], on_update=[]),
                        bass_nofuse=True,
                    )
                    nc.register_instruction(nop, overwrite=True)
                    new.append(nop)
                si.on_wait = waits[-max_waits:]
                inst.sync_info = si
            new.append(inst)
        if changed:
            blk.instructions = new


def _build_program():
    nc = bass.Bass("TRN2", target_bir_lowering=False, debug=False, num_devices=NCORES)

    xt = nc.dram_tensor("xt", [128, KC, TE], BF16, kind="ExternalInput")
    wq = nc.dram_tensor("wq", [KC, 128, KC, 128], BF16, kind="ExternalInput")
    wk = nc.dram_tensor("wk", [128, KC, KV], BF16, kind="ExternalInput")
    wv = nc.dram_tensor("wv", [128, KC, KV], BF16, kind="ExternalInput")
    wo = nc.dram_tensor("wo", [128, KC, C], BF16, kind="ExternalInput")
    bq = nc.dram_tensor("bq", [128, KC], F32, kind="ExternalInput")
    bk = nc.dram_tensor("bk", [128, 2], F32, kind="ExternalInput")
    ident = nc.dram_tensor("ident", [128, 128], BF16, kind="ExternalInput")
    # mask slots: 0 = kt0@qb0, 1 = kt1@qb0, 2 = kt0@qb>0 (T0), 3 = kt2 (T2)
    maskp = nc.dram_tensor("maskp", [128, 4, NG, 128], BF16, kind="ExternalInput")
    out = nc.dram_tensor("out", [TQ, C], BF16, kind="ExternalOutput")

    with _ChunkedDrainTileContext(nc) as tc:
        with contextlib.ExitStack() as ctx:
            wsb = ctx.enter_context(tc.tile_pool(name="wsb", bufs=1))
            xsb = ctx.enter_context(tc.tile_pool(name="xsb", bufs=1))
            csb = ctx.enter_context(tc.tile_pool(name="csb", bufs=1))
            qkv = ctx.enter_context(tc.tile_pool(name="qkv", bufs=1))
            ynp = ctx.enter_context(tc.tile_pool(name="ynp", bufs=1))
            expp = ctx.enter_context(tc.tile_pool(name="expp", bufs=6))
            rcpp = ctx.enter_context(tc.tile_pool(name="rcpp", bufs=4))
            ytnp = ctx.enter_context(tc.tile_pool(name="ytnp", bufs=4))
            obp = ctx.enter_context(tc.tile_pool(name="obp", bufs=2))
            pj = ctx.enter_context(tc.tile_pool(name="pj", bufs=1, space="PSUM"))
            scp_pool = ctx.enter_context(tc.tile_pool(name="scp", bufs=2, space="PSUM"))
            # combined av-output (fp32 [128,4,65]) + transpose-output (bf16
            # [128,2,128] via bitcast) tile: 1552B -> one psum bank
            ytp_pool = ctx.enter_context(tc.tile_pool(name="ytp", bufs=1, space="PSUM"))

            EARLY = [True]

            def proj_psum():
                # before the attention pipeline starts, projections borrow
                # the scores pool's big tiles (double-buffered); once units
                # are flowing they use the dedicated pj bank so they don't
                # steal the scores rotation.
                if EARLY[0]:
                    t = scp_pool.tile([128, 3, NG, 128], F32, name="scp", tag="sc")
                    return t[:, 0, :, :].rearrange("p g q -> p (g q)")
                return pj.tile([128, 512], F32, name="pp", tag="pj")

            # ---- loads, ordered by consumption deadline ----
            wk_sb = wsb.tile([128, KC, KV], BF16, name="wk_sb", tag="wk")
            nc.sync.dma_start(out=wk_sb[:], in_=wk[:])
            xt_sb = xsb.tile([128, KC, TE], BF16, name="xt_sb", tag="xt")
            nc.sync.dma_start(out=xt_sb[:, 0:4, 0:384], in_=xt[:, 0:4, 0:384])
            nc.sync.dma_start(out=xt_sb[:, 4:8, 0:384], in_=xt[:, 4:8, 0:384])
            bk_sb = csb.tile([128, 2], F32)
            nc.sync.dma_start(out=bk_sb[:], in_=bk[:])
            wv_sb = wsb.tile([128, KC, KV], BF16, name="wv_sb", tag="wv")
            nc.sync.dma_start(out=wv_sb[:], in_=wv[:])
            nc.sync.dma_start(out=xt_sb[:, :, 384:TE], in_=xt[:, :, 384:TE])
            bq_sb = csb.tile([128, KC], F32)
            nc.sync.dma_start(out=bq_sb[:], in_=bq[:])

            wq_sb = [None] * KC

            def _load_wq(m):
                t = wsb.tile([128, KC, 128], BF16, name=f"wq{m}", tag=f"wq{m}")
                nc.sync.dma_start(out=t[:], in_=wq[m, :, :, :])
                wq_sb[m] = t

            for m in range(2):
                _load_wq(m)
            id_sb = csb.tile([128, 128], BF16)
            nc.sync.dma_start(out=id_sb[:], in_=ident[:])
            mask_sb = csb.tile([128, 4, NG, 128], BF16)
            nc.sync.dma_start(out=mask_sb[:], in_=maskp[:])
            for m in range(2, KC):
                _load_wq(m)
            wo_sb = wsb.tile([128, KC, C], BF16, name="wo_sb", tag="wo")
            nc.sync.dma_start(out=wo_sb[:, :, 0:512], in_=wo[:, :, 0:512])
            nc.sync.dma_start(out=wo_sb[:, :, 512:1024], in_=wo[:, :, 512:1024])

            # ---- kT projection: kT4 [64, NG, TE] bf16 (base-partition 0;
            # bf16 matmuls with operands at base partition 64 crash walrus)
            kT4 = qkv.tile([64, NG, TE], BF16, name="kT4", tag="kT4")

            def k_proj(s2, mt):
                kp = proj_psum()
                for kc in range(KC):
                    nc.tensor.matmul(
                        kp[:, 0:384],
                        wk_sb[:, kc, mt * 128 : (mt + 1) * 128],
                        xt_sb[:, kc, s2 * 384 : (s2 + 1) * 384],
                        start=(kc == 0),
                        stop=(kc == KC - 1),
                    )
                for gh in range(2):
                    g = 2 * mt + gh
                    if gh == 0:
                        nc.vector.tensor_scalar_add(
                            kT4[0:64, g, s2 * 384 : (s2 + 1) * 384],
                            kp[0:64, 0:384],
                            bk_sb[0:64, mt : mt + 1],
                        )
                    else:
                        nc.scalar.activation(
                            kT4[0:64, g, s2 * 384 : (s2 + 1) * 384],
                            kp[64:128, 0:384],
                            mybir.ActivationFunctionType.Identity,
                            bias=bk_sb[64:128, mt : mt + 1],
                        )

            # ---- v projection: token-major [128, NG, 65] with ones column ----
            v_sb = []
            for vt in range(6):
                t = qkv.tile([128, NG, HD + 1], BF16, name=f"v{vt}", tag=f"v{vt}")
                nc.vector.memset(t[:, :, HD : HD + 1], 1.0)
                v_sb.append(t)

            def v_proj(vt):
                vp = proj_psum()
                for kc in range(KC):
                    nc.tensor.matmul(
                        vp[:, 0:KV],
                        xt_sb[:, kc, vt * 128 : (vt + 1) * 128],
                        wv_sb[:, kc, :],
                        start=(kc == 0),
                        stop=(kc == KC - 1),
                    )
                nc.scalar.copy(
                    v_sb[vt][:, :, 0:HD],
                    vp[:, 0:KV].rearrange("p (g d) -> p g d", g=NG),
                )

            yn = ynp.tile([128, KC, TQ], BF16)
            qT_sb = [None] * KC

            def q_proj(m):
                qp = proj_psum()
                for kc in range(KC):
                    nc.tensor.matmul(
                        qp[:],
                        wq_sb[m][:, kc, :],
                        xt_sb[:, kc, WINDOW:TE],
                        start=(kc == 0),
                        stop=(kc == KC - 1),
                    )
                qT = qkv.tile([64, 2, TQ], BF16, name=f"qT{m}", tag=f"qT{m}")
                nc.vector.tensor_scalar_add(
                    qT[0:64, 0, :], qp[0:64, :], bq_sb[0:64, m : m + 1]
                )
                nc.scalar.activation(
                    qT[0:64, 1, :],
                    qp[64:128, :],
                    mybir.ActivationFunctionType.Identity,
                    bias=bq_sb[64:128, m : m + 1],
                )
                qT_sb[m] = qT

            def attn_scores(g, qb):
                """12 score matmuls + exp + masks; returns masked ex tile."""
                scp = scp_pool.tile([128, 3, NG, 128], F32, name="scp", tag="sc")
                for kt in range(3):
                    ke0 = qb * 128 + kt * 128
                    for mi in range(2):
                        m = 2 * g + mi
                        # 2 heads per matmul: rhs [64, 2, 128] (hi = 2*mi+hh)
                        nc.tensor.matmul(
                            scp[:, kt, 2 * mi : 2 * mi + 2, :],
                            kT4[0:64, g, ke0 : ke0 + 128],
                            qT_sb[m][0:64, :, qb * 128 : qb * 128 + 128],
                            start=True,
                            stop=True,
                        )
                ex = expp.tile([128, 3, NG, 128], BF16, name="ex", tag="ex")
                nc.scalar.activation(ex[:], scp[:], mybir.ActivationFunctionType.Exp)
                return ex

            def attn_masks(g, qb, ex):
                # band masks (multiplicative 0/1): kt0 always, kt1 only at qb0,
                # kt2 always. Middle tile fully valid for qb>0. Emitted AFTER
                # the previous unit's recip/norm so they don't head-of-line
                # block the DVE queue while waiting on exp.
                # edge cores: qb0 kt0/kt1 fully padded; qb1 kt0 also reaches
                # into the padding (slot 0 = T0 on std cores, zeros on edge)
                slot0 = 0 if qb <= 1 else 2
                nc.gpsimd.tensor_tensor(
                    ex[:, 0, :, :], ex[:, 0, :, :], mask_sb[:, slot0, :, :],
                    mybir.AluOpType.mult,
                )
                if qb == 0:
                    nc.gpsimd.tensor_tensor(
                        ex[:, 1, :, :], ex[:, 1, :, :], mask_sb[:, 1, :, :],
                        mybir.AluOpType.mult,
                    )
                nc.vector.tensor_tensor(
                    ex[:, 2, :, :], ex[:, 2, :, :], mask_sb[:, 3, :, :],
                    mybir.AluOpType.mult,
                )

            def attn_av(g, qb, ex):
                """av (transposed) + recip + norm. kt order 1,2,0 so the
                unmasked middle tile starts immediately after exp and the
                slow gpsimd kt0 mask gets maximal slack."""
                comb = ytp_pool.tile([128, 388], F32, name="comb", tag="yt")
                y_t = comb[:, 0:260].rearrange("p (h d) -> p h d", h=NG)
                # hi outer: only one psum accumulation group open per bank
                for hi in range(4):
                    for kt in (1, 2, 0):
                        nc.tensor.matmul(
                            y_t[:, hi, :],
                            ex[:, kt, hi, :],
                            v_sb[qb + kt][:, g, :],
                            start=(kt == 1),
                            stop=(kt == 0),
                        )
                rcp = rcpp.tile([128, NG, 1], F32, name="rcp", tag="rcp")
                with nc.allow_low_precision(reason="softmax denom reciprocal"):
                    nc.vector.reciprocal(rcp[:], y_t[:, :, HD : HD + 1])
                y_n = ytnp.tile([128, NG, HD], BF16, name="y_n", tag="y_n")
                if USE_BCAST:
                    nc.vector.tensor_tensor(
                        y_n[:],
                        y_t[:, :, 0:HD],
                        rcp[:].broadcast_to([128, NG, HD]),
                        mybir.AluOpType.mult,
                    )
                else:
                    for hi in range(4):
                        nc.vector.tensor_scalar_mul(
                            y_n[:, hi, :], y_t[:, hi, 0:HD], rcp[:, hi, :]
                        )
                return comb, y_n

            def attn_tr(g, qb, comb, y_n):
                """transpose back to head-major (bf16 psum region of comb)."""
                if USE_IS_TRANSPOSE:
                    yTp = comb[:, 260:388].bitcast(BF16).rearrange("p (m q) -> p m q", m=2)
                    for mi in range(2):
                        nc.tensor.matmul(
                            yTp[:, mi, :],
                            y_n[:, 2 * mi : 2 * mi + 2, :],
                            id_sb[:],
                            start=True,
                            stop=True,
                            is_transpose=True,
                        )
                else:
                    yTf = pj.tile([128, 512], F32, name="pp", tag="pj")
                    yTp = yTf[:, 0:256].rearrange("p (m q) -> p m q", m=2)
                    for mi in range(2):
                        nc.tensor.matmul(
                            yTp[:, mi, :],
                            y_n[:, 2 * mi : 2 * mi + 2, :],
                            id_sb[:],
                            start=True,
                            stop=True,
                        )
                nc.vector.tensor_copy(
                    yn[:, 2 * g : 2 * g + 2, qb * 128 : qb * 128 + 128], yTp[:]
                )

            ob_cur = [None]

            def out_chunk(tt, n2):
                if n2 == 0:
                    ob_cur[0] = obp.tile([128, C], BF16, name="ob", tag="ob")
                ob = ob_cur[0]
                op = pj.tile([128, 512], F32, name="op", tag="pj")
                for m in range(KC):
                    nc.tensor.matmul(
                        op[:],
                        yn[:, m, tt * 128 : (tt + 1) * 128],
                        wo_sb[:, m, n2 * 512 : (n2 + 1) * 512],
                        start=(m == 0),
                        stop=(m == KC - 1),
                    )
                nc.vector.tensor_copy(ob[:, n2 * 512 : (n2 + 1) * 512], op[:])
                if n2 == 1:
                    nc.sync.dma_start(out=out[tt * 128 : (tt + 1) * 128, :], in_=ob[:])

            # ---- schedule ----
            # unit (g, qb) pipeline with lag-1 av and lag-2 transpose;
            # proj chunks as filler at the head of each unit (they absorb the
            # exp->scores psum-reuse wait).
            # mid-fillers sit between av(prev) and tr(prev) to absorb the
            # DVE recip/norm latency with PE work.
            mid = {
                (1, 1): [lambda: out_chunk(0, 0)],
                (2, 1): [lambda: out_chunk(0, 1)],
                (1, 2): [lambda: out_chunk(1, 0)],
                (2, 2): [lambda: out_chunk(1, 1)],
                (1, 3): [lambda: out_chunk(2, 0)],
                (2, 3): [lambda: out_chunk(2, 1)],
            }
            pend = None  # (g, qb, ex) awaiting av+tr

            # unit pipeline: scores(u) | av(prev)+recip/norm | masks(u) |
            # filler | transpose(prev). qb0 interleaves the q/v projections;
            # qb>=1 uses out-proj chunks as mid fillers.
            mid_f = {
                (1, 1): [lambda: out_chunk(0, 0)],
                (2, 1): [lambda: out_chunk(0, 1)],
                (1, 2): [lambda: out_chunk(1, 0)],
                (2, 2): [lambda: out_chunk(1, 1)],
                (1, 3): [lambda: out_chunk(2, 0)],
                (2, 3): [lambda: out_chunk(2, 1)],
            }
            PHASE = int(os.environ.get("KV2_PHASE", "5"))
            k_proj(0, 0)
            k_proj(0, 1)
            v_proj(0)
            v_proj(1)
            v_proj(2)
            k_proj(1, 0)
            k_proj(1, 1)
            q_proj(0)
            q_proj(1)
            if PHASE == 1:
                for m in range(2, KC):
                    q_proj(m)
                nc.sync.dma_start(out=out[0:128, 0:512], in_=qT_sb[0][:])
            for g in (range(NG) if PHASE >= 2 else []):
                ex = attn_scores(g, 0)
                if g < 3:
                    q_proj(2 * g + 2)
                if PHASE >= 3 and pend is not None:
                    comb, y_n = attn_av(*pend)
                    attn_masks(g, 0, ex)
                    if g < 3:
                        q_proj(2 * g + 3)
                    v_proj(g + 2)
                    attn_tr(pend[0], pend[1], comb, y_n)
                else:
                    attn_masks(g, 0, ex)
                    if g == 0:
                        q_proj(3)
                    elif g < 3:
                        q_proj(2 * g + 3)
                        v_proj(g + 2)
                pend = (g, 0, ex)
            if PHASE == 2:
                nc.sync.dma_start(out=out[0:128, 0:512], in_=pend[2][:, 0, :, :].rearrange("p a b -> p (a b)"))
            EARLY[0] = False
            if PHASE == 3:
                comb, y_n = attn_av(*pend)
                attn_tr(pend[0], pend[1], comb, y_n)
                nc.sync.dma_start(out=out[0:128, 0:512], in_=yn[:, 0, :])
            for qb in (range(1, 4) if PHASE >= 4 else []):
                for g in range(NG):
                    u = (g, qb)
                    ex = attn_scores(*u)
                    comb, y_n = attn_av(*pend)
                    attn_masks(u[0], u[1], ex)
                    if PHASE >= 5:
                        for f in mid_f.get(u, []):
                            f()
                    attn_tr(pend[0], pend[1], comb, y_n)
                    pend = (u[0], u[1], ex)
            if PHASE >= 4:
                comb, y_n = attn_av(*pend)
                attn_tr(pend[0], pend[1], comb, y_n)
            if PHASE == 4:
                nc.sync.dma_start(out=out[0:128, 0:512], in_=yn[:, 0, :])
            if PHASE >= 5:
                out_chunk(3, 0)
                out_chunk(3, 1)

    _split_multi_waits(nc)
    return nc


_NC = None


def _get_nc():
    global _NC
    if _NC is None:
        _NC = _build_program()
    return _NC


def _to_bf16(a):
    import ml_dtypes

    return np.asarray(a, np.float32).astype(ml_dtypes.bfloat16)


def _to_fp8(a):
    from concourse import mybir as _mb

    return np.asarray(a, np.float32).astype(_mb.dt.np(_mb.dt.float8e4))


def _host_prep(x, Wq, bq, Wk, bk, Wv, bv, Wo, bo):
    x = np.asarray(x, np.float32)
    Wq = np.asarray(Wq, np.float32)
    bq = np.asarray(bq, np.float32)
    Wk = np.asarray(Wk, np.float32)
    bk = np.asarray(bk, np.float32)
    Wv = np.asarray(Wv, np.float32)
    bv = np.asarray(bv, np.float32)
    Wo = np.asarray(Wo, np.float32)
    bo = np.asarray(bo, np.float32)

    scale = np.float32(1.0 / np.sqrt(HD))
    # wq[m][p, kc, n] = Wq[kc*128+p, m*128+n] * scale
    wq_t = _to_bf16(
        np.ascontiguousarray((Wq * scale).reshape(KC, 128, KC, 128).transpose(2, 1, 0, 3))
    )
    wk_t = _to_bf16(np.ascontiguousarray(Wk.reshape(KC, 128, KV).transpose(1, 0, 2)))
    wv_t = _to_bf16(np.ascontiguousarray(Wv.reshape(KC, 128, KV).transpose(1, 0, 2)))
    wo_t = _to_bf16(np.ascontiguousarray(Wo.reshape(KC, 128, C).transpose(1, 0, 2)))
    # bq_sb[p, m] = bq[m*128+p] * scale
    bq_t = np.ascontiguousarray((bq * scale).reshape(KC, 128).T)
    # bk_sb[gh*64+d, mt] = bk[(2*mt+gh)*64+d]
    bk_t = np.ascontiguousarray(bk.reshape(2, 2, 64).transpose(1, 2, 0).reshape(128, 2))
    ident = _to_bf16(np.eye(128, dtype=np.float32))

    # masks [128 kj, slot, g(replicated), 128 qi]
    kj = np.arange(128)[:, None]
    qi = np.arange(128)[None, :]
    t0 = (kj >= qi).astype(np.float32)  # kt0 (and qb0-interior)
    t2 = (kj <= qi).astype(np.float32)  # kt2
    ones = np.ones((128, 128), np.float32)
    zeros = np.zeros((128, 128), np.float32)

    def mk_mask(edge):
        # slots: 0 = kt0@qb0, 1 = kt1@qb0, 2 = kt0@qb>0, 3 = kt2
        s0 = zeros if edge else t0
        s1 = zeros if edge else ones
        m = np.stack([s0, s1, t0, t2])  # [4, 128, 128]
        return _to_bf16(np.broadcast_to(m[None, :, :, :], (NG, 4, 128, 128)).transpose(2, 1, 0, 3).copy())

    mask_std = mk_mask(False)
    mask_edge = mk_mask(True)

    in_maps = []
    for core in range(NCORES):
        b, c = core // 4, core % 4
        t0c = c * TQ - WINDOW
        xe = np.zeros((TE, C), np.float32)
        lo = max(t0c, 0)
        xe[lo - t0c : TE, :] = x[b, lo : t0c + TE, :]
        xt_t = _to_bf16(np.ascontiguousarray(xe.T.reshape(KC, 128, TE).transpose(1, 0, 2)))
        in_maps.append(
            {
                "xt": xt_t,
                "wq": wq_t,
                "wk": wk_t,
                "wv": wv_t,
                "wo": wo_t,
                "bq": bq_t,
                "bk": bk_t,
                "ident": ident,
                "maskp": mask_edge if c == 0 else mask_std,
            }
        )

    bv_rep = np.concatenate([bv[(h // NG) * HD : (h // NG + 1) * HD] for h in range(NH)])
    corr = bv_rep.astype(np.float64) @ Wo.astype(np.float64) + bo.astype(np.float64)
    return in_maps, corr.astype(np.float32)


LAST_RESULTS = None


def kernel(x, Wq, bq, Wk, bk, Wv, bv, Wo, bo):
    global LAST_RESULTS
    in_maps, corr = _host_prep(x, Wq, bq, Wk, bk, Wv, bv, Wo, bo)
    nc = _get_nc()
    res = run_bass_kernel_spmd(nc, in_maps, core_ids=list(range(NCORES)))
    LAST_RESULTS = res
    out = np.empty((B, T, C), np.float32)
    for core in range(NCORES):
        b, c = core // 4, core % 4
        out[b, c * TQ : (c + 1) * TQ, :] = res.results[core]["out"].astype(np.float32)
    out += corr[None, None, :]
    return out


# revision 4
# speedup vs baseline: 1.0725x; 1.0595x over previous
"""Trainium2 Bass kernel for GQA causal sliding-window self-attention.

Problem: B=2, T=2048, C=1024, 16 heads (hd=64), 4 KV groups, window=256.

Sharding: data-parallel over (batch, T-chunk): 8 cores = 2 batches x 4
chunks of 512 query tokens; each core gets a 768-token extended x slice
(512 queries + 256 halo) and computes its output rows locally.

Design: all-bf16 operands (halves DMA, 1 cyc/col matmuls at any free
size), diagonal 128-query attention blocks (384-key span = 1.5x
overcompute instead of 2x), 2-head-packed score matmuls (qT stored
[64, 2, TQ] per head-pair, kT [64, NG, TE], all matmul operands at base
partition 0), TRANSPOSED av (queries on partitions; ones column in v
gives softmax denominators) so the reciprocal is per-partition and
normalization is one stride-0-broadcast DVE multiply, then a transpose
back to head-major via a plain bf16 matmul against an identity matrix.
Band masks are multiplicative 0/1 bf16 constants (GPSIMD kt0 +
qb0-edge kt1; DVE kt2; the middle kt tile needs no mask), applied to
the exp'd scores. Output is bf16, upcast on host; host folds 1/8 into
Wq/bq and adds the exact linear bv/bo correction (bv_rep @ Wo + bo).

PSUM (8 banks): scores [128,3,4,128] x2 bufs (6), combined
y_t[128,4,65]+bf16 transpose region x1 (1), shared proj/out bank (1).
Early projections borrow the scores pool before attention starts.

Pipeline: per unit (group, 128-query block): scores -> exp (ScalarE)
-> masks -> av -> recip/norm (DVE) -> transpose -> yn copy, with
q/v-projection and out-projection chunks interleaved as PE filler.

Environment constraints found the hard way (this walrus/axon build):
 - max 1 sync wait per instruction (_split_multi_waits hoists extras
   onto same-engine NOPs)
 - bf16 matmuls with operands at base partition 64 crash the compile;
   all operands must sit at partition 0 (hence qT/kT layouts)
 - is_transpose=True matmuls return wrong data on hw (interp-only)
 - only one open PSUM accumulation group per 2KB bank
 - GPSIMD cannot touch PSUM; DVE/ScalarE cross-partition-offset ok
 - DMA cannot read PSUM
"""

import sys

sys.path.insert(0, "/opt/trn_rl_repo")

import contextlib

import numpy as np

import concourse.bass as bass
import concourse.tile as tile
from concourse import mybir
from concourse.bass_utils import run_bass_kernel_spmd
from concourse.vector_clock import ScopedClock

F32 = mybir.dt.float32
BF16 = mybir.dt.bfloat16
F8E4 = mybir.dt.float8e4
FP8_SCALE = 32.0  # weights *32 into fp8 normal range; q carries the 1/32

import os

USE_IS_TRANSPOSE = os.environ.get("KV2_ISTR", "") == "1"  # wrong results on hw
USE_BCAST = os.environ.get("KV2_NO_BCAST", "") != "1"
USE_SC_IDBIAS = os.environ.get("KV2_NO_IDBIAS", "") != "1"

B, T, C = 2, 2048, 1024
NH, NG, HD = 16, 4, 64
KV = NG * HD  # 256
WINDOW = 256
NCORES = 8
TQ = 512  # query tokens per core
TE = TQ + WINDOW  # 768 extended tokens per core
KC = C // 128  # 8 contraction tiles


class _ChunkedDrainTileContext(tile.TileContext):
    """Walrus in this container only accepts 1 sync wait on CTRL-class
    instructions; spread the tail drain waits over engine NOPs."""

    def _drain_and_barrier(self, tick_clock, wait_clock):
        gc = tick_clock.global_clock
        entries = []
        for scope, vc in ScopedClock({None: gc}).items():
            for proc in range(len(vc)):
                t = vc[proc]
                if t > 0:
                    entries.append((scope, proc, t))
        engines = [self.nc.sync, self.nc.vector, self.nc.scalar, self.nc.gpsimd]
        curs = [ScopedClock() for _ in engines]
        for i, (scope, proc, t) in enumerate(entries):
            eng = engines[i % len(engines)]
            nop = eng.nop(nofuse=True, hint="tail_wait")
            partial = ScopedClock()
            partial.require_at_least(scope, proc, t)
            wait_clock.add_sem_waits(nop.ins, partial, curs[i % len(engines)])
            curs[i % len(engines)].update_past(partial)
        self.nc.all_engine_barrier(sem_only=True)
        drain_inst = self.nc.sync.drain()
        cur = ScopedClock()
        for c in curs:
            cur.update_past(c)
        wait_clock.add_sem_waits(drain_inst.ins, ScopedClock({None: gc}), cur)
        assert self.sems is not None
        popped = self.nc._tile_sem_poison_stack.pop()
        assert popped is self._sem_poison
        self.nc.clear_and_free_semaphores(list(self.sems.allocated().values()))


def _split_multi_waits(nc, max_waits=1):
    """Hoist excess sync waits onto same-engine NOPs (walrus quirk)."""
    fn = nc.m.functions[0]
    for blk in fn.blocks:
        insts = blk.instructions
        new = []
        changed = False
        for inst in insts:
            si = inst.sync_info
            waits = list(si.on_wait) if si is not None and si.on_wait else []
            if len(waits) > max_waits:
                changed = True
                for w in waits[:-max_waits]:
                    nop = mybir.InstNoOp(
                        name=nc.get_next_instruction_name(),
                        ins=[],
                        outs=[],
                        engine=inst.engine,
                        sync_info=mybir.SyncInfo(on_wait=[w], on_update=[]),
                        bass_nofuse=True,
                    )
                    nc.register_instruction(nop, overwrite=True)
                    new.append(nop)
                si.on_wait = waits[-max_waits:]
                inst.sync_info = si
            new.append(inst)
        if changed:
            blk.instructions = new


def _build_program():
    nc = bass.Bass("TRN2", target_bir_lowering=False, debug=False, num_devices=NCORES)

    xt = nc.dram_tensor("xt", [128, KC, TE], BF16, kind="ExternalInput")
    wq = nc.dram_tensor("wq", [KC, 128, KC, 128], BF16, kind="ExternalInput")
    wk = nc.dram_tensor("wk", [128, KC, KV], BF16, kind="ExternalInput")
    wv = nc.dram_tensor("wv", [128, KC, KV], BF16, kind="ExternalInput")
    wo = nc.dram_tensor("wo", [128, KC, C], BF16, kind="ExternalInput")
    bq = nc.dram_tensor("bq", [128, KC], F32, kind="ExternalInput")
    bk = nc.dram_tensor("bk", [128, 2], F32, kind="ExternalInput")
    ident = nc.dram_tensor("ident", [128, 128], BF16, kind="ExternalInput")
    # mask slots: 0 = kt0@qb0, 1 = kt1@qb0, 2 = kt0@qb>0 (T0), 3 = kt2 (T2)
    maskp = nc.dram_tensor("maskp", [128, 4, NG, 128], BF16, kind="ExternalInput")
    out = nc.dram_tensor("out", [TQ, C], BF16, kind="ExternalOutput")

    with _ChunkedDrainTileContext(nc) as tc:
        with contextlib.ExitStack() as ctx:
            wsb = ctx.enter_context(tc.tile_pool(name="wsb", bufs=1))
            xsb = ctx.enter_context(tc.tile_pool(name="xsb", bufs=1))
            csb = ctx.enter_context(tc.tile_pool(name="csb", bufs=1))
            qkv = ctx.enter_context(tc.tile_pool(name="qkv", bufs=1))
            ynp = ctx.enter_context(tc.tile_pool(name="ynp", bufs=1))
            expp = ctx.enter_context(tc.tile_pool(name="expp", bufs=6))
            rcpp = ctx.enter_context(tc.tile_pool(name="rcpp", bufs=4))
            ytnp = ctx.enter_context(tc.tile_pool(name="ytnp", bufs=4))
            obp = ctx.enter_context(tc.tile_pool(name="obp", bufs=2))
            pj = ctx.enter_context(tc.tile_pool(name="pj", bufs=1, space="PSUM"))
            scp_pool = ctx.enter_context(tc.tile_pool(name="scp", bufs=2, space="PSUM"))
            # combined av-output (fp32 [128,4,65]) + transpose-output (bf16
            # [128,2,128] via bitcast) tile: 1552B -> one psum bank
            ytp_pool = ctx.enter_context(tc.tile_pool(name="ytp", bufs=1, space="PSUM"))

            EARLY = [True]

            def proj_psum():
                # before the attention pipeline starts, projections borrow
                # the scores pool's big tiles (double-buffered); once units
                # are flowing they use the dedicated pj bank so they don't
                # steal the scores rotation.
                if EARLY[0]:
                    t = scp_pool.tile([128, 3, NG, 128], F32, name="scp", tag="sc")
                    return t[:, 0, :, :].rearrange("p g q -> p (g q)")
                return pj.tile([128, 512], F32, name="pp", tag="pj")

            # ---- loads, ordered by consumption deadline ----
            wk_sb = wsb.tile([128, KC, KV], BF16, name="wk_sb", tag="wk")
            nc.sync.dma_start(out=wk_sb[:], in_=wk[:])
            xt_sb = xsb.tile([128, KC, TE], BF16, name="xt_sb", tag="xt")
            nc.sync.dma_start(out=xt_sb[:, 0:4, 0:384], in_=xt[:, 0:4, 0:384])
            nc.sync.dma_start(out=xt_sb[:, 4:8, 0:384], in_=xt[:, 4:8, 0:384])
            bk_sb = csb.tile([128, 2], F32)
            nc.sync.dma_start(out=bk_sb[:], in_=bk[:])
            wv_sb = wsb.tile([128, KC, KV], BF16, name="wv_sb", tag="wv")
            nc.sync.dma_start(out=wv_sb[:], in_=wv[:])
            nc.sync.dma_start(out=xt_sb[:, :, 384:TE], in_=xt[:, :, 384:TE])
            bq_sb = csb.tile([128, KC], F32)
            nc.sync.dma_start(out=bq_sb[:], in_=bq[:])

            wq_sb = [None] * KC

            def _load_wq(m):
                t = wsb.tile([128, KC, 128], BF16, name=f"wq{m}", tag=f"wq{m}")
                nc.sync.dma_start(out=t[:], in_=wq[m, :, :, :])
                wq_sb[m] = t

            for m in range(2):
                _load_wq(m)
            id_sb = csb.tile([128, 128], BF16)
            nc.sync.dma_start(out=id_sb[:], in_=ident[:])
            mask_sb = csb.tile([128, 4, NG, 128], BF16)
            nc.sync.dma_start(out=mask_sb[:], in_=maskp[:])
            for m in range(2, KC):
                _load_wq(m)
            wo_sb = wsb.tile([128, KC, C], BF16, name="wo_sb", tag="wo")
            nc.sync.dma_start(out=wo_sb[:, :, 0:512], in_=wo[:, :, 0:512])
            nc.sync.dma_start(out=wo_sb[:, :, 512:1024], in_=wo[:, :, 512:1024])

            # ---- kT projection: kT4 [64, NG, TE] bf16 (base-partition 0;
            # bf16 matmuls with operands at base partition 64 crash walrus)
            kT4 = qkv.tile([64, NG, TE], BF16, name="kT4", tag="kT4")

            def k_proj(s2, mt):
                kp = proj_psum()
                for kc in range(KC):
                    nc.tensor.matmul(
                        kp[:, 0:384],
                        wk_sb[:, kc, mt * 128 : (mt + 1) * 128],
                        xt_sb[:, kc, s2 * 384 : (s2 + 1) * 384],
                        start=(kc == 0),
                        stop=(kc == KC - 1),
                    )
                for gh in range(2):
                    g = 2 * mt + gh
                    if gh == 0:
                        nc.vector.tensor_scalar_add(
                            kT4[0:64, g, s2 * 384 : (s2 + 1) * 384],
                            kp[0:64, 0:384],
                            bk_sb[0:64, mt : mt + 1],
                        )
                    else:
                        nc.scalar.activation(
                            kT4[0:64, g, s2 * 384 : (s2 + 1) * 384],
                            kp[64:128, 0:384],
                            mybir.ActivationFunctionType.Identity,
                            bias=bk_sb[64:128, mt : mt + 1],
                        )

            # ---- v projection: token-major [128, NG, 65] with ones column ----
            v_sb = []
            for vt in range(6):
                t = qkv.tile([128, NG, HD + 1], BF16, name=f"v{vt}", tag=f"v{vt}")
                nc.vector.memset(t[:, :, HD : HD + 1], 1.0)
                v_sb.append(t)

            def v_proj(vt):
                vp = proj_psum()
                for kc in range(KC):
                    nc.tensor.matmul(
                        vp[:, 0:KV],
                        xt_sb[:, kc, vt * 128 : (vt + 1) * 128],
                        wv_sb[:, kc, :],
                        start=(kc == 0),
                        stop=(kc == KC - 1),
                    )
                nc.scalar.copy(
                    v_sb[vt][:, :, 0:HD],
                    vp[:, 0:KV].rearrange("p (g d) -> p g d", g=NG),
                )

            yn = ynp.tile([128, KC, TQ], BF16)
            qT_sb = [None] * KC

            def q_proj(m):
                qp = proj_psum()
                for kc in range(KC):
                    nc.tensor.matmul(
                        qp[:],
                        wq_sb[m][:, kc, :],
                        xt_sb[:, kc, WINDOW:TE],
                        start=(kc == 0),
                        stop=(kc == KC - 1),
                    )
                qT = qkv.tile([64, 2, TQ], BF16, name=f"qT{m}", tag=f"qT{m}")
                nc.vector.tensor_scalar_add(
                    qT[0:64, 0, :], qp[0:64, :], bq_sb[0:64, m : m + 1]
                )
                nc.scalar.activation(
                    qT[0:64, 1, :],
                    qp[64:128, :],
                    mybir.ActivationFunctionType.Identity,
                    bias=bq_sb[64:128, m : m + 1],
                )
                qT_sb[m] = qT

            def attn_scores(g, qb):
                """12 score matmuls + exp + masks; returns masked ex tile."""
                scp = scp_pool.tile([128, 3, NG, 128], F32, name="scp", tag="sc")
                for kt in range(3):
                    ke0 = qb * 128 + kt * 128
                    for mi in range(2):
                        m = 2 * g + mi
                        # 2 heads per matmul: rhs [64, 2, 128] (hi = 2*mi+hh)
                        nc.tensor.matmul(
                            scp[:, kt, 2 * mi : 2 * mi + 2, :],
                            kT4[0:64, g, ke0 : ke0 + 128],
                            qT_sb[m][0:64, :, qb * 128 : qb * 128 + 128],
                            start=True,
                            stop=True,
                        )
                ex = expp.tile([128, 3, NG, 128], BF16, name="ex", tag="ex")
                nc.scalar.activation(ex[:], scp[:], mybir.ActivationFunctionType.Exp)
                return ex

            def attn_masks(g, qb, ex):
                # band masks (multiplicative 0/1): kt0 always, kt1 only at qb0,
                # kt2 always. Middle tile fully valid for qb>0. Emitted AFTER
                # the previous unit's recip/norm so they don't head-of-line
                # block the DVE queue while waiting on exp.
                # edge cores: qb0 kt0/kt1 fully padded; qb1 kt0 also reaches
                # into the padding (slot 0 = T0 on std cores, zeros on edge)
                slot0 = 0 if qb <= 1 else 2
                nc.vector.tensor_tensor(
                    ex[:, 0, :, :], ex[:, 0, :, :], mask_sb[:, slot0, :, :],
                    mybir.AluOpType.mult,
                )
                if qb == 0:
                    nc.gpsimd.tensor_tensor(
                        ex[:, 1, :, :], ex[:, 1, :, :], mask_sb[:, 1, :, :],
                        mybir.AluOpType.mult,
                    )
                nc.vector.tensor_tensor(
                    ex[:, 2, :, :], ex[:, 2, :, :], mask_sb[:, 3, :, :],
                    mybir.AluOpType.mult,
                )

            def attn_av(g, qb, ex):
                """av (transposed) + recip + norm. kt order 1,2,0 so the
                unmasked middle tile starts immediately after exp and the
                slow gpsimd kt0 mask gets maximal slack."""
                comb = ytp_pool.tile([128, 388], F32, name="comb", tag="yt")
                y_t = comb[:, 0:260].rearrange("p (h d) -> p h d", h=NG)
                # hi outer: only one psum accumulation group open per bank
                for hi in range(4):
                    for kt in (1, 0, 2):
                        nc.tensor.matmul(
                            y_t[:, hi, :],
                            ex[:, kt, hi, :],
                            v_sb[qb + kt][:, g, :],
                            start=(kt == 1),
                            stop=(kt == 2),
                        )

                rcp = rcpp.tile([128, NG, 1], F32, name="rcp", tag="rcp")
                with nc.allow_low_precision(reason="softmax denom reciprocal"):
                    nc.vector.reciprocal(rcp[:], y_t[:, :, HD : HD + 1])
                y_n = ytnp.tile([128, NG, HD], BF16, name="y_n", tag="y_n")
                if USE_BCAST:
                    nc.vector.tensor_tensor(
                        y_n[:],
                        y_t[:, :, 0:HD],
                        rcp[:].broadcast_to([128, NG, HD]),
                        mybir.AluOpType.mult,
                    )
                else:
                    for hi in range(4):
                        nc.vector.tensor_scalar_mul(
                            y_n[:, hi, :], y_t[:, hi, 0:HD], rcp[:, hi, :]
                        )
                return comb, y_n

            def attn_tr(g, qb, comb, y_n):
                """transpose back to head-major (bf16 psum region of comb)."""
                if USE_IS_TRANSPOSE:
                    yTp = comb[:, 260:388].bitcast(BF16).rearrange("p (m q) -> p m q", m=2)
                    for mi in range(2):
                        nc.tensor.matmul(
                            yTp[:, mi, :],
                            y_n[:, 2 * mi : 2 * mi + 2, :],
                            id_sb[:],
                            start=True,
                            stop=True,
                            is_transpose=True,
                        )
                else:
                    yTf = pj.tile([128, 512], F32, name="pp", tag="pj")
                    yTp = yTf[:, 0:256].rearrange("p (m q) -> p m q", m=2)
                    for mi in range(2):
                        nc.tensor.matmul(
                            yTp[:, mi, :],
                            y_n[:, 2 * mi : 2 * mi + 2, :],
                            id_sb[:],
                            start=True,
                            stop=True,
                        )
                nc.vector.tensor_copy(
                    yn[:, 2 * g : 2 * g + 2, qb * 128 : qb * 128 + 128], yTp[:]
                )

            ob_cur = [None]

            def out_chunk(tt, n2):
                if n2 == 0:
                    ob_cur[0] = obp.tile([128, C], BF16, name="ob", tag="ob")
                ob = ob_cur[0]
                op = pj.tile([128, 512], F32, name="op", tag="pj")
                for m in range(KC):
                    nc.tensor.matmul(
                        op[:],
                        yn[:, m, tt * 128 : (tt + 1) * 128],
                        wo_sb[:, m, n2 * 512 : (n2 + 1) * 512],
                        start=(m == 0),
                        stop=(m == KC - 1),
                    )
                nc.vector.tensor_copy(ob[:, n2 * 512 : (n2 + 1) * 512], op[:])
                if n2 == 1:
                    nc.sync.dma_start(out=out[tt * 128 : (tt + 1) * 128, :], in_=ob[:])

            # ---- schedule ----
            # unit (g, qb) pipeline with lag-1 av and lag-2 transpose;
            # proj chunks as filler at the head of each unit (they absorb the
            # exp->scores psum-reuse wait).
            # mid-fillers sit between av(prev) and tr(prev) to absorb the
            # DVE recip/norm latency with PE work.
            mid = {
                (1, 1): [lambda: out_chunk(0, 0)],
                (2, 1): [lambda: out_chunk(0, 1)],
                (1, 2): [lambda: out_chunk(1, 0)],
                (2, 2): [lambda: out_chunk(1, 1)],
                (1, 3): [lambda: out_chunk(2, 0)],
                (2, 3): [lambda: out_chunk(2, 1)],
            }
            pend = None  # (g, qb, ex) awaiting av+tr

            # unit pipeline: scores(u) | av(prev)+recip/norm | masks(u) |
            # filler | transpose(prev). qb0 interleaves the q/v projections;
            # qb>=1 uses out-proj chunks as mid fillers.
            mid_f = {
                (1, 1): [lambda: out_chunk(0, 0)],
                (2, 1): [lambda: out_chunk(0, 1)],
                (1, 2): [lambda: out_chunk(1, 0)],
                (2, 2): [lambda: out_chunk(1, 1)],
                (1, 3): [lambda: out_chunk(2, 0)],
                (2, 3): [lambda: out_chunk(2, 1)],
            }
            PHASE = int(os.environ.get("KV2_PHASE", "5"))
            k_proj(0, 0)
            k_proj(0, 1)
            v_proj(0)
            v_proj(1)
            v_proj(2)
            k_proj(1, 0)
            k_proj(1, 1)
            q_proj(0)
            q_proj(1)
            if PHASE == 1:
                for m in range(2, KC):
                    q_proj(m)
                nc.sync.dma_start(out=out[0:128, 0:512], in_=qT_sb[0][:])
            for g in (range(NG) if PHASE >= 2 else []):
                ex = attn_scores(g, 0)
                if g < 3:
                    q_proj(2 * g + 2)
                if PHASE >= 3 and pend is not None:
                    comb, y_n = attn_av(*pend)
                    attn_masks(g, 0, ex)
                    if g < 3:
                        q_proj(2 * g + 3)
                    v_proj(g + 2)
                    attn_tr(pend[0], pend[1], comb, y_n)
                else:
                    attn_masks(g, 0, ex)
                    if g == 0:
                        q_proj(3)
                    elif g < 3:
                        q_proj(2 * g + 3)
                        v_proj(g + 2)
                pend = (g, 0, ex)
            if PHASE == 2:
                nc.sync.dma_start(out=out[0:128, 0:512], in_=pend[2][:, 0, :, :].rearrange("p a b -> p (a b)"))
            EARLY[0] = False
            if PHASE == 3:
                comb, y_n = attn_av(*pend)
                attn_tr(pend[0], pend[1], comb, y_n)
                nc.sync.dma_start(out=out[0:128, 0:512], in_=yn[:, 0, :])
            for qb in (range(1, 4) if PHASE >= 4 else []):
                for g in range(NG):
                    u = (g, qb)
                    ex = attn_scores(*u)
                    comb, y_n = attn_av(*pend)
                    attn_masks(u[0], u[1], ex)
                    if PHASE >= 5:
                        for f in mid_f.get(u, []):
                            f()
                    attn_tr(pend[0], pend[1], comb, y_n)
                    pend = (u[0], u[1], ex)
            if PHASE >= 4:
                comb, y_n = attn_av(*pend)
                attn_tr(pend[0], pend[1], comb, y_n)
            if PHASE == 4:
                nc.sync.dma_start(out=out[0:128, 0:512], in_=yn[:, 0, :])
            if PHASE >= 5:
                out_chunk(3, 0)
                out_chunk(3, 1)

    _split_multi_waits(nc)
    return nc


_NC = None


def _get_nc():
    global _NC
    if _NC is None:
        _NC = _build_program()
    return _NC


def _to_bf16(a):
    import ml_dtypes

    return np.asarray(a, np.float32).astype(ml_dtypes.bfloat16)


def _to_fp8(a):
    from concourse import mybir as _mb

    return np.asarray(a, np.float32).astype(_mb.dt.np(_mb.dt.float8e4))


def _host_prep(x, Wq, bq, Wk, bk, Wv, bv, Wo, bo):
    x = np.asarray(x, np.float32)
    Wq = np.asarray(Wq, np.float32)
    bq = np.asarray(bq, np.float32)
    Wk = np.asarray(Wk, np.float32)
    bk = np.asarray(bk, np.float32)
    Wv = np.asarray(Wv, np.float32)
    bv = np.asarray(bv, np.float32)
    Wo = np.asarray(Wo, np.float32)
    bo = np.asarray(bo, np.float32)

    scale = np.float32(1.0 / np.sqrt(HD))
    # wq[m][p, kc, n] = Wq[kc*128+p, m*128+n] * scale
    wq_t = _to_bf16(
        np.ascontiguousarray((Wq * scale).reshape(KC, 128, KC, 128).transpose(2, 1, 0, 3))
    )
    wk_t = _to_bf16(np.ascontiguousarray(Wk.reshape(KC, 128, KV).transpose(1, 0, 2)))
    wv_t = _to_bf16(np.ascontiguousarray(Wv.reshape(KC, 128, KV).transpose(1, 0, 2)))
    wo_t = _to_bf16(np.ascontiguousarray(Wo.reshape(KC, 128, C).transpose(1, 0, 2)))
    # bq_sb[p, m] = bq[m*128+p] * scale
    bq_t = np.ascontiguousarray((bq * scale).reshape(KC, 128).T)
    # bk_sb[gh*64+d, mt] = bk[(2*mt+gh)*64+d]
    bk_t = np.ascontiguousarray(bk.reshape(2, 2, 64).transpose(1, 2, 0).reshape(128, 2))
    ident = _to_bf16(np.eye(128, dtype=np.float32))

    # masks [128 kj, slot, g(replicated), 128 qi]
    kj = np.arange(128)[:, None]
    qi = np.arange(128)[None, :]
    t0 = (kj >= qi).astype(np.float32)  # kt0 (and qb0-interior)
    t2 = (kj <= qi).astype(np.float32)  # kt2
    ones = np.ones((128, 128), np.float32)
    zeros = np.zeros((128, 128), np.float32)

    def mk_mask(edge):
        # slots: 0 = kt0@qb0, 1 = kt1@qb0, 2 = kt0@qb>0, 3 = kt2
        s0 = zeros if edge else t0
        s1 = zeros if edge else ones
        m = np.stack([s0, s1, t0, t2])  # [4, 128, 128]
        return _to_bf16(np.broadcast_to(m[None, :, :, :], (NG, 4, 128, 128)).transpose(2, 1, 0, 3).copy())

    mask_std = mk_mask(False)
    mask_edge = mk_mask(True)

    in_maps = []
    for core in range(NCORES):
        b, c = core // 4, core % 4
        t0c = c * TQ - WINDOW
        xe = np.zeros((TE, C), np.float32)
        lo = max(t0c, 0)
        xe[lo - t0c : TE, :] = x[b, lo : t0c + TE, :]
        xt_t = _to_bf16(np.ascontiguousarray(xe.T.reshape(KC, 128, TE).transpose(1, 0, 2)))
        in_maps.append(
            {
                "xt": xt_t,
                "wq": wq_t,
                "wk": wk_t,
                "wv": wv_t,
                "wo": wo_t,
                "bq": bq_t,
                "bk": bk_t,
                "ident": ident,
                "maskp": mask_edge if c == 0 else mask_std,
            }
        )

    bv_rep = np.concatenate([bv[(h // NG) * HD : (h // NG + 1) * HD] for h in range(NH)])
    corr = bv_rep.astype(np.float64) @ Wo.astype(np.float64) + bo.astype(np.float64)
    return in_maps, corr.astype(np.float32)


LAST_RESULTS = None


def kernel(x, Wq, bq, Wk, bk, Wv, bv, Wo, bo):
    global LAST_RESULTS
    in_maps, corr = _host_prep(x, Wq, bq, Wk, bk, Wv, bv, Wo, bo)
    nc = _get_nc()
    res = run_bass_kernel_spmd(nc, in_maps, core_ids=list(range(NCORES)))
    LAST_RESULTS = res
    out = np.empty((B, T, C), np.float32)
    for core in range(NCORES):
        b, c = core // 4, core % 4
        out[b, c * TQ : (c + 1) * TQ, :] = res.results[core]["out"].astype(np.float32)
    out += corr[None, None, :]
    return out
